# revision 1
# baseline (speedup 1.0000x reference)
"""Trainium2 Bass kernel for 2-layer GraphSAGE (mean aggregation) on 8 NeuronCores.

Strategy (graph/data parallel, dst-partitioned):
  - Destination nodes sharded across 8 cores (12.5K each); edges partitioned by
    destination core and sorted by (dest-tile, src-quarter) on the host.
  - Source features gathered from DRAM quarter-tables with SWDGE dma_gather
    (int16 indices, 512B rows), 4 SWDGE queues round-robin.
  - Segment-mean computed as a one-hot mask matmul on the TensorEngine with
    float32r (full-rate fp32): aggT[f, d] += g_chunk.T @ mask, where
    mask[e, d] = (iota[d] == dst_local[e]) * invdeg[e] is built by one
    DVE tensor_scalar op per 128-edge chunk.
  - Dense branch h = agg @ Wl.T + x @ Wr.T done feature-major; x^T obtained by
    PE-transposing the core's own shard rows.
  - Between layers: AllGather of the h shard into 4 shared quarter-tables so
    layer 2 can gather remote rows; quarter-wise AG overlaps layer-2 compute.
"""

import numpy as np

# ---------------------------------------------------------------- problem dims
N_NODES = 100000
N_EDGES = 800000
D = 128
NC = 8

TILE_D = 512          # destination-tile width (matmul moving free dim)
STILE_SIZES = (7, 6, 6, 6)  # dest-tiles per gather super-group
GATHER_BUFS = 4
NQ = 4                # SWDGE queues

_cache = {}


# ------------------------------------------------------------------- host plan
def _plan(edge_index, n_nodes, n_cores, tile_d, stile_sizes):
    """Partition + sort edges, compute padded per-group layouts shared by all
    cores, and build per-core index/mask streams."""
    src = np.asarray(edge_index[0], dtype=np.int64)
    dst = np.asarray(edge_index[1], dtype=np.int64)
    E = src.shape[0]

    nloc = n_nodes // n_cores
    quart = nloc // 4
    tbl_rows = quart * n_cores
    nt = -(-nloc // tile_d)               # tiles per core
    nloc_pad = nt * tile_d

    # stile partition of tiles
    stiles = []
    t0 = 0
    for s in stile_sizes:
        stiles.append(list(range(t0, min(t0 + s, nt))))
        t0 += s
    stiles = [s for s in stiles if s]
    assert sum(len(s) for s in stiles) == nt

    deg = np.bincount(dst, minlength=n_nodes).astype(np.float64)
    invdeg = (1.0 / np.maximum(deg, 1.0)).astype(np.float32)

    core = dst // nloc
    dloc = dst % nloc
    tile = dloc // tile_d
    dtl = (dloc % tile_d).astype(np.float32)
    srcm = src % nloc
    chunk = srcm // quart
    tblrow = (src // nloc) * quart + srcm % quart   # row within quarter-table
    assert tblrow.max() < tbl_rows

    # group id = (core, tile, chunk); count group sizes
    gid = (core * nt + tile) * 4 + chunk
    order = np.argsort(gid, kind="stable")
    counts = np.bincount(gid, minlength=n_cores * nt * 4).reshape(n_cores, nt, 4)
    # padded group sizes, shared across cores
    gmax = counts.max(axis=0)                       # [nt, 4]
    gpad = ((gmax + 127) // 128) * 128              # multiple of 128
    ep = int(gpad.sum())                            # padded stream length/core

    # stream position of each group, in (stile, chunk, tile) order
    goff = np.zeros((nt, 4), dtype=np.int64)
    pos = 0
    call_list = []                                  # (chunk, [tiles], off, n)
    for tiles in stiles:
        for c in range(4):
            call_off = pos
            for t in tiles:
                goff[t, c] = pos
                pos += int(gpad[t, c])
            call_list.append((c, tiles, call_off, pos - call_off))
    assert pos == ep

    # per-core streams
    idx_st = np.zeros((n_cores, ep), dtype=np.int16)
    dst_st = np.full((n_cores, ep), -1.0, dtype=np.float32)
    inv_st = np.zeros((n_cores, ep), dtype=np.float32)

    gid_s = gid[order]
    put = np.empty(E, dtype=np.int64)
    # position of each sorted edge inside its group
    grp_start = np.searchsorted(gid_s, np.arange(n_cores * nt * 4))
    within = np.arange(E) - grp_start[gid_s]
    k_s = gid_s // (nt * 4)
    t_s = (gid_s // 4) % nt
    c_s = gid_s % 4
    put = goff[t_s, c_s] + within
    idx_st[k_s, put] = tblrow[order].astype(np.int16)
    dst_st[k_s, put] = dtl[order]
    inv_st[k_s, put] = invdeg[dst[order]]

    return dict(
        nloc=nloc, quart=quart, tbl_rows=tbl_rows, nt=nt, nloc_pad=nloc_pad,
        stiles=stiles, gpad=gpad, goff=goff, ep=ep, call_list=call_list,
        idx_st=idx_st, dst_st=dst_st, inv_st=inv_st, tile_d=tile_d,
        n_cores=n_cores, n_nodes=n_nodes,
    )


def _wrap16(stream):
    """[ep] -> [128, ep//16] wrapped-16 + replicated layout for dma_gather."""
    ep = stream.shape[0]
    w = stream.reshape(ep // 16, 16).T          # [16, ep//16]
    return np.tile(w, (8, 1))                   # [128, ep//16]


def _colmajor(stream):
    """[ep] -> [128, ep//128] with element j at [j%128, j//128]."""
    ep = stream.shape[0]
    return stream.reshape(ep // 128, 128).T.copy()


def _make_tables(x_full, plan):
    """x rows -> 4 quarter-tables with row (n) -> table[(n%nloc)//quart],
    row (n//nloc)*quart + (n%nloc)%quart."""
    nloc, quart, tbl = plan["nloc"], plan["quart"], plan["tbl_rows"]
    n_cores = plan["n_cores"]
    tabs = []
    xr = x_full.reshape(n_cores, nloc, D)
    for q in range(4):
        tabs.append(np.ascontiguousarray(
            xr[:, q * quart:(q + 1) * quart, :].reshape(tbl, D)))
    return tabs


# --------------------------------------------------------------- bass builder
def _build(plan, iters=1):
    import os
    SKIP_AG = bool(int(os.environ.get("K_SKIP_AG", "0")))
    SKIP_MASK = bool(int(os.environ.get("K_SKIP_MASK", "0")))
    SKIP_MM = bool(int(os.environ.get("K_SKIP_MM", "0")))
    SKIP_DENSE = bool(int(os.environ.get("K_SKIP_DENSE", "0")))
    MASK_MODE = int(os.environ.get("K_MASK_MODE", "0"))
    import concourse.bass as bass
    import concourse.tile as tile
    from concourse import bacc, mybir
    from concourse.library_config import mlp
    from concourse.tile_rust import add_dep_helper

    f32 = mybir.dt.float32
    f32r = mybir.dt.float32r
    i16 = mybir.dt.int16

    nloc = plan["nloc"]; quart = plan["quart"]; tbl = plan["tbl_rows"]
    nt = plan["nt"]; nloc_pad = plan["nloc_pad"]; td = plan["tile_d"]
    ep = plan["ep"]; gpad = plan["gpad"]; goff = plan["goff"]
    call_list = plan["call_list"]; stiles = plan["stiles"]
    n_cores = plan["n_cores"]
    nblk = td // 128                     # 128-blocks per dest tile
    max_call = max(n for (_, _, _, n) in call_list)

    nc = bacc.Bacc("TRN2", target_bir_lowering=False, debug=False,
                   num_swdge_queues=NQ)

    # inputs
    x_tabs = [nc.dram_tensor(f"x_tab{q}", [tbl, D], f32, kind="ExternalInput")
              for q in range(4)]
    x_shard = nc.dram_tensor("x_shard", [nloc_pad, D], f32, kind="ExternalInput")
    idxs_in = nc.dram_tensor("idxs", [128, ep // 16], i16, kind="ExternalInput")
    dstv_in = nc.dram_tensor("dstv", [128, ep // 128], f32, kind="ExternalInput")
    invv_in = nc.dram_tensor("invv", [128, ep // 128], f32, kind="ExternalInput")
    iota_in = nc.dram_tensor("iota", [128, td], f32, kind="ExternalInput")
    ident_in = nc.dram_tensor("ident", [128, 128], f32, kind="ExternalInput")
    w_in = {}
    for nm in ("w1lt", "w1rt", "w2lt", "w2rt"):
        w_in[nm] = nc.dram_tensor(nm, [128, 128], f32, kind="ExternalInput")
    b_in = {nm: nc.dram_tensor(nm, [128, 1], f32, kind="ExternalInput")
            for nm in ("b1", "b2")}
    out_t = nc.dram_tensor("outT", [128, nloc_pad], f32, kind="ExternalOutput")

    # internal DRAM
    h_shard = nc.dram_tensor("h_shard", [nloc_pad, D], f32)
    h_tabs = [nc.dram_tensor(f"h_tab{q}", [tbl, D], f32, addr_space="Shared")
              for q in range(4)]

    with tile.TileContext(nc) as tc:
        lib_inst = nc.gpsimd.load_library(mlp)
        with (
            tc.tile_pool(name="persist", bufs=1) as pp,
            tc.tile_pool(name="gather", bufs=GATHER_BUFS) as gpo,
            tc.tile_pool(name="mask", bufs=3) as mpo,
            tc.tile_pool(name="aggT", bufs=max(len(s) for s in stiles) + 1) as apo,
            tc.tile_pool(name="small", bufs=2) as spo,
            tc.tile_pool(name="psA", bufs=2, space="PSUM") as psa,
            tc.tile_pool(name="psB", bufs=2, space="PSUM") as psb,
            tc.tile_pool(name="psC", bufs=2, space="PSUM") as psc,
        ):
            # persistent SBUF
            idx_sb = pp.tile([128, ep // 16], i16)
            nc.sync.dma_start(idx_sb[:], idxs_in[:])
            dstv_sb = pp.tile([128, ep // 128], f32)
            nc.sync.dma_start(dstv_sb[:], dstv_in[:])
            invv_sb = pp.tile([128, ep // 128], f32)
            nc.sync.dma_start(invv_sb[:], invv_in[:])
            iota_sb = pp.tile([128, td], f32)
            nc.sync.dma_start(iota_sb[:], iota_in[:])
            ident_sb = pp.tile([128, 128], f32)
            nc.sync.dma_start(ident_sb[:], ident_in[:])
            w_sb = {}
            for nm, t in w_in.items():
                w_f = pp.tile([128, 128], f32, tag=nm + "f", name=f"wf_{nm}")
                nc.sync.dma_start(w_f[:], t[:])
                w_sb[nm] = pp.tile([128, 128], f32r, tag=nm, name=f"w_{nm}")
                nc.vector.tensor_copy(w_sb[nm][:], w_f[:])
            b_sb = {}
            for nm, t in b_in.items():
                b_sb[nm] = pp.tile([128, 1], f32, tag=nm, name=f"b_{nm}")
                nc.sync.dma_start(b_sb[nm][:], t[:])

            first_gather = [True]
            const_m = pp.tile([128, td], f32r, tag="constm", name="constm")
            nc.vector.tensor_copy(const_m[:], iota_sb[:])

            def layer(src_tabs, self_src, wl, wr, bias, is_last, ag_insts):
                """Emit one SAGE layer. Returns list of h-store instructions."""
                store_insts = []
                for tiles in stiles:
                    # issue the stile's 4 gather calls (parallel queues)
                    gbufs = {}
                    ginsts = {}
                    for (c, ctiles, off, n) in call_list:
                        if ctiles is not tiles:
                            continue
                        g = gpo.tile([128, max_call // 128, D], f32r, tag="g")
                        slots = n // 128
                        gi = nc.gpsimd.dma_gather(
                            g[:, :slots, :], src_tabs[c][:].bitcast(f32r),
                            idx_sb[:, off // 16:(off + n) // 16],
                            n, n, D, queue_num=c,
                            single_packet=False)
                        if first_gather[0]:
                            add_dep_helper(gi.ins, lib_inst.ins, sync=True,
                                           reason="lib before gather")
                            first_gather[0] = False
                        if ag_insts is not None:
                            add_dep_helper(gi.ins, ag_insts[c].ins, sync=True,
                                           reason="gather after AG")
                        gbufs[c] = (g, off)
                        ginsts[c] = gi
                    # consume chunk-major: per chunk, all its tiles' groups
                    aggs = {}
                    for c in range(4):
                        g, off = gbufs[c]
                        for t in tiles:
                            npads = int(gpad[t, c])
                            if npads == 0:
                                continue
                            base = int(goff[t, c]) - off      # within call
                            ps = psa.tile([128, td], f32, tag="psagg")
                            nchunks = npads // 128
                            for ci in range(nchunks):
                                col = (off + base) // 128 + ci
                                if SKIP_MASK:
                                    m = const_m
                                else:
                                    mdt = f32 if MASK_MODE in (1, 3) else f32r
                                    m = mpo.tile([128, td], mdt, tag="m")
                                    s1 = dstv_sb[:, col:col + 1] if MASK_MODE in (0, 1) else 0.5
                                    s2 = invv_sb[:, col:col + 1] if MASK_MODE in (0, 1) else 0.25
                                    nc.vector.tensor_scalar(
                                        m[:], iota_sb[:], s1, s2,
                                        mybir.AluOpType.is_equal,
                                        mybir.AluOpType.mult)
                                if not SKIP_MM:
                                    nc.tensor.matmul(
                                        ps[:],
                                        g[:, base // 128 + ci, :],
                                        m[:],
                                        start=(ci == 0), stop=(ci == nchunks - 1))
                            if SKIP_MM:
                                continue
                            if t not in aggs:
                                aggs[t] = apo.tile([128, td], f32r, tag="agg", name=f"agg_t{t}")
                                nc.vector.tensor_copy(aggs[t][:], ps[:])
                            else:
                                nc.vector.tensor_add(aggs[t][:], aggs[t][:], ps[:])
                    # dense + activation + store per tile
                    for t in tiles:
                        if SKIP_DENSE:
                            continue
                        if t not in aggs:
                            aggs[t] = apo.tile([128, td], f32r, tag="agg", name=f"agg_t{t}")
                            nc.vector.tensor_copy(aggs[t][:], const_m[:])
                        # self rows -> selfT via PE transpose
                        xr = spo.tile([128, nblk, 128], f32, tag="xr")
                        nc.sync.dma_start(
                            xr[:],
                            self_src[t * td:(t + 1) * td, :].rearrange(
                                "(a p) f -> p a f", p=128))
                        selfT = spo.tile([128, td], f32r, tag="selfT")
                        for a in range(nblk):
                            tp = psb.tile([128, 128], f32, tag="pst")
                            nc.tensor.transpose(tp[:], xr[:, a, :], ident_sb[:])
                            nc.scalar.copy(selfT[:, a * 128:(a + 1) * 128], tp[:])
                        hp = psc.tile([128, td], f32, tag="psh")
                        agg_t = aggs.get(t)
                        nc.tensor.matmul(hp[:], wl[:], agg_t[:],
                                         start=True, stop=False)
                        nc.tensor.matmul(hp[:], wr[:], selfT[:],
                                         start=False, stop=True)
                        hT = spo.tile([128, td], f32, tag="hT")
                        if is_last:
                            nc.vector.tensor_scalar_add(hT[:], hp[:], bias[:])
                            si = nc.sync.dma_start(
                                out_t[:, t * td:(t + 1) * td], hT[:])
                            store_insts.append(si)
                        else:
                            sg = spo.tile([128, td], f32, tag="sg")
                            nc.scalar.activation(
                                sg[:], hp[:], mybir.ActivationFunctionType.Sigmoid,
                                bias=bias[:])
                            tmp = spo.tile([128, td], f32, tag="tmpb")
                            nc.vector.tensor_scalar_add(tmp[:], hp[:], bias[:])
                            nc.vector.tensor_mul(hT[:], tmp[:], sg[:])
                            # transpose back to row-major and store to h_shard
                            hr = spo.tile([128, nblk, 128], f32, tag="hr")
                            for a in range(nblk):
                                tp = psb.tile([128, 128], f32, tag="pst")
                                nc.tensor.transpose(
                                    tp[:], hT[:, a * 128:(a + 1) * 128],
                                    ident_sb[:])
                                nc.scalar.copy(hr[:, a, :], tp[:])
                            si = nc.sync.dma_start(
                                h_shard[t * td:(t + 1) * td, :].rearrange(
                                    "(a p) f -> p a f", p=128),
                                hr[:])
                            store_insts.append(si)
                return store_insts

            for _ in range(iters):
                l1_stores = layer(x_tabs, x_shard,
                                  w_sb["w1lt"], w_sb["w1rt"], b_sb["b1"],
                                  False, None)
                ag_insts = []
                if SKIP_AG:
                    ag_insts = None
                for q in range(4) if not SKIP_AG else []:
                    ag = nc.gpsimd.collective_compute(
                        "AllGather", mybir.AluOpType.bypass,
                        replica_groups=[list(range(n_cores))],
                        ins=[h_shard[q * quart:(q + 1) * quart, :]],
                        outs=[h_tabs[q][:]])
                    for si in l1_stores:
                        add_dep_helper(ag.ins, si.ins, sync=True,
                                       reason="AG after h stores")
                    ag_insts.append(ag)
                layer(h_tabs, h_shard,
                      w_sb["w2lt"], w_sb["w2rt"], b_sb["b2"],
                      True, ag_insts)

    nc.compile()
    return nc


# ------------------------------------------------------------------ reference-
# shaped entry point
def _prepare(edge_index):
    plan = _plan(edge_index, N_NODES, NC, TILE_D, STILE_SIZES)
    return plan


def _in_maps(plan, x, w1l, w1r, b1, w2l, w2r, b2):
    x = np.ascontiguousarray(np.asarray(x, dtype=np.float32))
    tabs = _make_tables(x, plan)
    nloc, nloc_pad = plan["nloc"], plan["nloc_pad"]
    n_cores = plan["n_cores"]
    td = plan["tile_d"]
    iota = np.broadcast_to(np.arange(td, dtype=np.float32), (128, td)).copy()
    ident = np.eye(128, dtype=np.float32)
    xr = x.reshape(n_cores, nloc, D)
    maps = []
    for k in range(n_cores):
        xs = np.zeros((nloc_pad, D), np.float32)
        xs[:nloc] = xr[k]
        m = {
            "x_shard": xs,
            "idxs": _wrap16(plan["idx_st"][k]),
            "dstv": _colmajor(plan["dst_st"][k]),
            "invv": _colmajor(plan["inv_st"][k]),
            "iota": iota, "ident": ident,
            "w1lt": np.ascontiguousarray(np.asarray(w1l, np.float32).T),
            "w1rt": np.ascontiguousarray(np.asarray(w1r, np.float32).T),
            "w2lt": np.ascontiguousarray(np.asarray(w2l, np.float32).T),
            "w2rt": np.ascontiguousarray(np.asarray(w2r, np.float32).T),
            "b1": np.asarray(b1, np.float32).reshape(128, 1),
            "b2": np.asarray(b2, np.float32).reshape(128, 1),
        }
        for q in range(4):
            m[f"x_tab{q}"] = tabs[q]
        maps.append(m)
    return maps


def _run(inputs, iters=1):
    """Compile (cached) and run; returns full [N, D] output."""
    from concourse.bass_utils import run_bass_kernel_spmd

    edge_index = np.asarray(inputs["edge_index"])
    key = ("k", iters, edge_index.shape[1])
    if key not in _cache:
        plan = _prepare(edge_index)
        nc = _build(plan, iters=iters)
        _cache[key] = (plan, nc)
    plan, nc = _cache[key]
    maps = _in_maps(plan, inputs["x"], inputs["W1_l"], inputs["W1_r"],
                    inputs["b1"], inputs["W2_l"], inputs["W2_r"], inputs["b2"])
    res = run_bass_kernel_spmd(nc, maps, core_ids=list(range(plan["n_cores"])))
    nloc = plan["nloc"]
    outs = [np.asarray(res.results[k]["outT"]).T[:nloc] for k in range(plan["n_cores"])]
    return np.concatenate(outs, axis=0)


def kernel(**inputs) -> np.ndarray:
    return _run(inputs, iters=1)



# revision 3
# speedup vs baseline: 1364.1078x; 1364.1078x over previous
"""Trainium2 Bass kernel for 2-layer GraphSAGE (mean aggregation) on 8 NeuronCores.

v2 design (dst-sharded, fp16 on-chip):
  - Destination nodes sharded across 8 cores (12.5K each); edges partitioned by
    (dest-tile, src-quarter) and sorted by local dst within each group.
  - Source features gathered from DRAM quarter-tables in fp16 (256B rows) with
    SWDGE dma_gather, 4 queues; indices sorted within each 128-edge chunk for
    DRAM locality.
  - Segment-sum via windowed one-hot mask matmuls: each 128-edge chunk only
    streams its dst window [o, o+w) (w ~ 64-128 instead of 512). A zero matmul
    opens each dest tile's PSUM accumulation (start=True over all 512 cols) and
    a closing zero matmul carries stop=True.
  - Masks are pure 0/1 built in fp16 with ONE broadcast tensor_tensor per
    (tile, quarter) group: (dstrel[:, chunk] == iota[w]). Mean normalization
    (1/deg) applied per dst column when evacuating PSUM (tensor_tensor mult
    with a host-replicated invdeg tile).
  - Dense branch: h^T = Wl@aggT + Wr@selfT; selfT comes from host-pretransposed
    xT (layer 1) or the SBUF-resident hT of layer 1 (layer 2) - no transposes
    on the self path. hT->row-major transposes only for the h table stores.
  - Between layers: per-quarter AllGather of fp16 h rows into 4 shared
    quarter-tables, each gated only on the stores of the tiles covering that
    quarter, so AG overlaps the tail of layer 1 and layer-2 gathers start per
    quarter.

Structure (chunk counts, windows, mask widths) is shared across the 8 cores
(SPMD single program): per-group chunk counts are cross-core maxima and chunk
windows are cross-core hulls; per-core streams are padded to match.
"""

import numpy as np

# ---------------------------------------------------------------- problem dims
N_NODES = 100000
N_EDGES = 800000
D = 128
NC = 8
import os as _os
TD = 512                  # dest-tile width
CH = 128                  # edges per chunk
STILE = int(_os.environ.get("K2_STILE", "3"))   # tiles per gather super-group
GATHER_BUFS = int(_os.environ.get("K2_GBUFS", "8"))
NQ = 4

_cache = {}


# ------------------------------------------------------------------- host plan
def _plan(edge_index, n_nodes, n_cores, td, stile):
    src = np.asarray(edge_index[0], dtype=np.int64)
    dst = np.asarray(edge_index[1], dtype=np.int64)
    E = src.shape[0]

    nloc = n_nodes // n_cores
    quart = nloc // 4
    tbl_rows = quart * n_cores
    nt = -(-nloc // td)
    nloc_pad = nt * td

    stiles = [list(range(s, min(s + stile, nt))) for s in range(0, nt, stile)]
    stile_of = np.zeros(nt, dtype=np.int64)
    for si, ts in enumerate(stiles):
        for t in ts:
            stile_of[t] = si

    deg = np.bincount(dst, minlength=n_nodes)
    invdeg = (1.0 / np.maximum(deg, 1.0)).astype(np.float32)
    assert deg.max() <= 50 * CH  # sanity

    core = dst // nloc
    dloc = dst % nloc
    t_of = dloc // td
    drel_all = dloc % td
    srcm = src % nloc
    c_of = srcm // quart
    tblrow = (src // nloc) * quart + srcm % quart
    assert tblrow.max() < tbl_rows < 2**15

    # per-core sort: (stile, c, t, drel)
    per_core = []
    counts = np.zeros((n_cores, nt, 4), dtype=np.int64)
    for k in range(n_cores):
        sel = np.flatnonzero(core == k)
        o = np.lexsort((drel_all[sel], t_of[sel], c_of[sel], stile_of[t_of[sel]]))
        sel = sel[o]
        per_core.append(sel)
        np.add.at(counts[k], (t_of[sel], c_of[sel]), 1)

    nch = -(-counts.max(axis=0) // CH)          # [nt, 4] shared chunk counts
    nch = np.maximum(nch, 1)                    # keep >=1 so start/stop exist

    # group slot offsets in (stile, c, t) order
    g_slot0 = np.zeros((nt, 4), dtype=np.int64)
    pos = 0
    call_list = []                              # (si, c, slot0, nslots, tiles)
    for si, ts in enumerate(stiles):
        for c in range(4):
            call0 = pos
            for t in ts:
                g_slot0[t, c] = pos
                pos += int(nch[t, c])
            call_list.append((si, c, call0, pos - call0, ts))
    S = pos                                     # total slots per core

    # per-core streams + per-chunk min/max for window hulls
    idx_st = np.zeros((n_cores, S * CH), dtype=np.int16)
    drel_st = np.full((n_cores, S * CH), -1.0, dtype=np.float32)
    wmin = np.full((n_cores, S), td, dtype=np.int64)
    wmax = np.full((n_cores, S), -1, dtype=np.int64)

    for k in range(n_cores):
        sel = per_core[k]
        t_k, c_k = t_of[sel], c_of[sel]
        # position within group
        gid = t_k * 4 + c_k
        # edges are sorted by (stile,c,t,drel) so group runs are contiguous
        ge = counts[k, t_k, c_k]
        # index within group via running position
        # compute start of each group's run in sel
        run_starts = {}
        within = np.empty(len(sel), dtype=np.int64)
        posk = 0
        # groups appear in (stile, c, t) order; iterate runs
        prev = None
        run0 = 0
        for i in range(len(sel)):
            key = (t_k[i], c_k[i])
            if key != prev:
                prev = key
                run0 = i
            within[i] = i - run0
        chunk = within // CH
        slot = g_slot0[t_k, c_k] + chunk
        lane = within % CH
        pos_in_stream = slot * CH + lane
        idx_st[k, pos_in_stream] = tblrow[sel].astype(np.int16)
        drel_st[k, pos_in_stream] = drel_all[sel]
        np.minimum.at(wmin[k], slot, drel_all[sel])
        np.maximum.at(wmax[k], slot, drel_all[sel])

        # sort each chunk's lanes by table row for DRAM locality
        ii = idx_st[k].reshape(S, CH)
        dd = drel_st[k].reshape(S, CH)
        # sort chunk lanes by table row for DRAM locality, pads last
        key = ii.astype(np.int64) + (dd < 0) * (1 << 20)
        order = np.argsort(key, axis=1, kind="stable")
        ii = np.take_along_axis(ii, order, axis=1)
        dd = np.take_along_axis(dd, order, axis=1)
        # point pad lanes at the chunk's last valid row (repeat gathers of the
        # same row are DRAM row-buffer hits, much cheaper than row 0 misses)
        nvalid = (dd >= 0).sum(axis=1)
        lastv = ii[np.arange(S), np.maximum(nvalid - 1, 0)]
        pad = dd < 0
        ii[pad] = np.broadcast_to(lastv[:, None], (S, CH))[pad]
        idx_st[k] = ii.reshape(-1)
        drel_st[k] = dd.reshape(-1)

    # window hulls (shared): even-aligned
    hmin = wmin.min(axis=0)
    hmax = wmax.max(axis=0)
    empty = hmax < 0                             # no core has edges in slot
    hmin[empty] = 0
    hmax[empty] = 0
    w_o = (hmin // 2) * 2
    w_end = np.minimum(((hmax + 2) // 2) * 2, td)
    w_w = np.maximum(w_end - w_o, 2)
    w_end = w_o + w_w

    # reorder each group's slots by descending window width (slot order is
    # structurally arbitrary) so mask builds can use tighter per-segment W
    perm = np.arange(S)
    for t in range(nt):
        for c in range(4):
            s0, n = int(g_slot0[t, c]), int(nch[t, c])
            if n > 1:
                ordg = np.argsort(-w_w[s0:s0 + n], kind="stable")
                perm[s0:s0 + n] = s0 + ordg
    w_o = w_o[perm]
    w_w = w_w[perm]
    for k in range(n_cores):
        ii = idx_st[k].reshape(S, CH)
        dd = drel_st[k].reshape(S, CH)
        idx_st[k] = ii[perm].reshape(-1)
        drel_st[k] = dd[perm].reshape(-1)

    # dstrel = drel - window offset of its chunk slot
    for k in range(n_cores):
        dd = drel_st[k].reshape(S, CH)
        dd_rel = dd - w_o[:, None]
        dd_rel[dd < 0] = -1.0
        drel_st[k] = dd_rel.reshape(-1)

    # per-group mask-build segments: slots are width-sorted descending, so
    # contiguous prefixes have monotone W. 2-way split minimizing cols.
    Wg = np.zeros((nt, 4), dtype=np.int64)
    segs = {}
    for t in range(nt):
        for c in range(4):
            s0, n = int(g_slot0[t, c]), int(nch[t, c])
            ws = w_w[s0:s0 + n]
            Wg[t, c] = ws.max() if n else 2
            best = [(0, n, int(ws.max()))]
            best_cost = n * int(ws.max())
            for j in range(1, n):
                cost = j * int(ws[0]) + (n - j) * int(ws[j])
                if cost < best_cost - 383:
                    best_cost = cost
                    best = [(0, j, int(ws[0])), (j, n - j, int(ws[j]))]
            segs[(t, c)] = best

    # invdeg broadcast tiles [128, nt*td] per core
    invb = np.ones((n_cores, nt * td), dtype=np.float32)
    for k in range(n_cores):
        invb[k, :nloc] = invdeg[k * nloc:(k + 1) * nloc]
    invb = np.broadcast_to(invb[:, None, :], (n_cores, 128, nt * td))

    return dict(
        nloc=nloc, quart=quart, tbl_rows=tbl_rows, nt=nt, nloc_pad=nloc_pad,
        stiles=stiles, call_list=call_list, S=S, nch=nch, g_slot0=g_slot0,
        segs=segs,
        w_o=w_o, w_w=w_w, Wg=Wg, idx_st=idx_st, drel_st=drel_st, invb=invb,
        td=td, n_cores=n_cores, n_nodes=n_nodes,
        max_call_slots=max(n for (_, _, _, n, _) in call_list),
        max_mask=int(max(int(nch[t, c]) * int(Wg[t, c])
                         for t in range(nt) for c in range(4))),
    )


def _wrap16(stream):
    ep = stream.shape[0]
    w = stream.reshape(ep // 16, 16).T
    return np.tile(w, (8, 1))


def _colmajor(stream):
    ep = stream.shape[0]
    return stream.reshape(ep // 128, 128).T.copy()


def _make_tables(x16, plan):
    nloc, quart, tbl = plan["nloc"], plan["quart"], plan["tbl_rows"]
    n_cores = plan["n_cores"]
    xr = x16.reshape(n_cores, nloc, D)
    return [np.ascontiguousarray(
        xr[:, q * quart:(q + 1) * quart, :].reshape(tbl, D)) for q in range(4)]


# --------------------------------------------------------------- bass builder
def _build(plan, iters=1):
    import os
    SKIP_AG = bool(int(os.environ.get("K_SKIP_AG", "0")))
    SKIP_MASK = bool(int(os.environ.get("K_SKIP_MASK", "0")))
    SKIP_MM = bool(int(os.environ.get("K_SKIP_MM", "0")))
    SKIP_DENSE = bool(int(os.environ.get("K_SKIP_DENSE", "0")))
    POOL_MASK = int(os.environ.get("K2_POOL_MASK", "0"))   # 1/N of masks on Pool
    ACT_EVAC = bool(int(os.environ.get("K2_ACT_EVAC", "1")))
    SINGLE_PACKET = bool(int(os.environ.get("K2_SP", "0")))
    import concourse.bass as bass
    import concourse.tile as tile
    from concourse import bacc, mybir
    from concourse.library_config import mlp
    from concourse.tile_rust import add_dep_helper

    f32 = mybir.dt.float32
    f16 = mybir.dt.float16
    i16 = mybir.dt.int16

    nloc = plan["nloc"]; quart = plan["quart"]; tbl = plan["tbl_rows"]
    nt = plan["nt"]; nloc_pad = plan["nloc_pad"]; td = plan["td"]
    S = plan["S"]; nch = plan["nch"]; g_slot0 = plan["g_slot0"]
    w_o = plan["w_o"]; w_w = plan["w_w"]; Wg = plan["Wg"]
    call_list = plan["call_list"]; stiles = plan["stiles"]
    n_cores = plan["n_cores"]
    nblk = td // 128
    max_call = plan["max_call_slots"]
    max_mask = plan["max_mask"]

    # tiles whose rows intersect quarter q (for AG gating)
    qtiles = [set(range(q * quart // td, ((q + 1) * quart - 1) // td + 1))
              for q in range(4)]

    nc = bacc.Bacc("TRN2", target_bir_lowering=False, debug=False,
                   num_swdge_queues=NQ)

    # inputs
    x_tabs = [nc.dram_tensor(f"x_tab{q}", [tbl, D], f16, kind="ExternalInput")
              for q in range(4)]
    xT_in = nc.dram_tensor("xT", [128, nloc_pad], f16, kind="ExternalInput")
    idxs_in = nc.dram_tensor("idxs", [128, S * 8], i16, kind="ExternalInput")
    drel_in = nc.dram_tensor("drel", [128, S], f16, kind="ExternalInput")
    invb_in = nc.dram_tensor("invb", [128, nt * td], f16, kind="ExternalInput")
    iota_in = nc.dram_tensor("iota", [128, td], f16, kind="ExternalInput")
    ident_in = nc.dram_tensor("ident", [128, 128], f16, kind="ExternalInput")
    w_in = {nm: nc.dram_tensor(nm, [128, 128], f16, kind="ExternalInput")
            for nm in ("w1lt", "w1rt", "w2lt", "w2rt")}
    b_in = {nm: nc.dram_tensor(nm, [128, 1], f32, kind="ExternalInput")
            for nm in ("b1", "b2")}
    out_t = nc.dram_tensor("outT", [128, nloc_pad], f16, kind="ExternalOutput")

    # internal DRAM
    h_shard = nc.dram_tensor("h_shard", [nloc_pad, D], f16)
    h_tabs = [nc.dram_tensor(f"h_tab{q}", [tbl, D], f16, addr_space="Shared")
              for q in range(4)]

    with tile.TileContext(nc) as tc:
        lib_inst = nc.gpsimd.load_library(mlp)
        with (
            tc.tile_pool(name="persist", bufs=1) as pp,
            tc.tile_pool(name="gather", bufs=GATHER_BUFS) as gpo,
            tc.tile_pool(name="mask", bufs=8) as mpo,
            tc.tile_pool(name="aggT", bufs=3) as apo,
            tc.tile_pool(name="small", bufs=3) as spo,
            tc.tile_pool(name="hrow", bufs=3) as hpo,
            tc.tile_pool(name="psA", bufs=2, space="PSUM") as psa,
            tc.tile_pool(name="psB", bufs=2, space="PSUM") as psb,
            tc.tile_pool(name="psC", bufs=2, space="PSUM") as psc,
        ):
            # persistent SBUF
            idx_sb = pp.tile([128, S * 8], i16)
            nc.sync.dma_start(idx_sb[:], idxs_in[:])
            drel_sb = pp.tile([128, S], f16)
            nc.sync.dma_start(drel_sb[:], drel_in[:])
            invb_sb = pp.tile([128, nt * td], f16)
            nc.sync.dma_start(invb_sb[:], invb_in[:])
            iota_sb = pp.tile([128, td], f16)
            nc.sync.dma_start(iota_sb[:], iota_in[:])
            ident_sb = pp.tile([128, 128], f16)
            nc.sync.dma_start(ident_sb[:], ident_in[:])
            zeros_sb = pp.tile([128, td], f16, tag="zeros", name="zeros")
            nc.vector.memset(zeros_sb[:], 0.0)
            w_sb = {}
            for nm, t_ in w_in.items():
                w_sb[nm] = pp.tile([128, 128], f16, tag=nm, name=f"w_{nm}")
                nc.sync.dma_start(w_sb[nm][:], t_[:])
            b_sb = {}
            for nm, t_ in b_in.items():
                b_sb[nm] = pp.tile([128, 1], f32, tag=nm, name=f"b_{nm}")
                nc.sync.dma_start(b_sb[nm][:], t_[:])
            hT_res = pp.tile([128, nt * td], f16, tag="hT", name="hT_res")
            xT_res = pp.tile([128, nloc_pad], f16, tag="xT", name="xT_res")
            nc.sync.dma_start(xT_res[:], xT_in[:])

            first_gather = [True]

            def layer(src_tabs, wl, wr, bias, is_last, ag_insts):
                """Emit one SAGE layer. Returns (store_insts per tile,
                gather insts)."""
                store_by_tile = {}
                gather_insts = []
                for si, ts in enumerate(stiles):
                    gbufs = {}
                    for (csi, c, call0, nslots, cts) in call_list:
                        if csi != si:
                            continue
                        g = gpo.tile([128, max_call, D], f16, tag="g")
                        n = nslots * CH
                        gi = nc.gpsimd.dma_gather(
                            g[:, :nslots, :], src_tabs[c][:],
                            idx_sb[:, call0 * 8:(call0 + nslots) * 8],
                            n, n, D, queue_num=c, single_packet=SINGLE_PACKET)
                        if first_gather[0]:
                            add_dep_helper(gi.ins, lib_inst.ins, sync=True,
                                           reason="lib before gather")
                            first_gather[0] = False
                        if ag_insts is not None:
                            add_dep_helper(gi.ins, ag_insts[c].ins, sync=True,
                                           reason="gather after AG")
                        gbufs[c] = (g, call0)
                        gather_insts.append(gi)
                    # per tile in this stile: masks, matmuls, dense
                    for t in ts:
                        ps = psa.tile([128, td], f32, tag="psagg")
                        if not SKIP_MM:
                            nc.tensor.matmul(ps[:], zeros_sb[:, :128],
                                             zeros_sb[:], start=True, stop=False)
                        for c in range(4):
                            g, call0 = gbufs[c]
                            s0 = int(g_slot0[t, c]); n_ch = int(nch[t, c])
                            W = int(Wg[t, c])
                            if SKIP_MASK:
                                m = None
                            else:
                                m = mpo.tile([128, n_ch, W], f16, tag="m")
                                eng = nc.vector
                                if POOL_MASK and (t * 4 + c) % POOL_MASK == 0:
                                    eng = nc.gpsimd
                                for (i0, nseg, Ws) in plan["segs"][(t, c)]:
                                    d_b = drel_sb[:, s0 + i0:s0 + i0 + nseg]\
                                        .broadcast_to((128, nseg, Ws))
                                    i_b = iota_sb[:, :Ws].rearrange(
                                        "p (o w) -> p o w", o=1).broadcast_to(
                                        (128, nseg, Ws))
                                    eng.tensor_tensor(
                                        m[:, i0:i0 + nseg, :Ws], d_b, i_b,
                                        mybir.AluOpType.is_equal)
                            if SKIP_MM:
                                continue
                            for i in range(n_ch):
                                sl = s0 + i
                                o, w = int(w_o[sl]), int(w_w[sl])
                                rhs = (m[:, i, :w] if m is not None
                                       else zeros_sb[:, :w])
                                nc.tensor.matmul(
                                    ps[:, o:o + w],
                                    g[:, sl - call0, :], rhs,
                                    start=False, stop=False)
                        if not SKIP_MM:
                            nc.tensor.matmul(ps[:], zeros_sb[:, :128],
                                             zeros_sb[:], start=False, stop=True)
                        if SKIP_DENSE:
                            continue
                        # aggT = ps * invdeg (per dst column), cast to fp16
                        aggT = apo.tile([128, td], f16, tag="agg")
                        if ACT_EVAC:
                            tmp = apo.tile([128, td], f16, tag="aggtmp")
                            nc.scalar.copy(tmp[:], ps[:])
                            nc.vector.tensor_mul(
                                aggT[:], tmp[:],
                                invb_sb[:, t * td:(t + 1) * td])
                        else:
                            nc.vector.tensor_mul(
                                aggT[:], ps[:],
                                invb_sb[:, t * td:(t + 1) * td])
                        # selfT
                        if is_last:
                            selfT = hT_res[:, t * td:(t + 1) * td]
                        else:
                            selfT = xT_res[:, t * td:(t + 1) * td]
                        hp = psc.tile([128, td], f32, tag="psh")
                        nc.tensor.matmul(hp[:], wl[:], aggT[:],
                                         start=True, stop=False)
                        nc.tensor.matmul(hp[:], wr[:], selfT,
                                         start=False, stop=True)
                        if is_last:
                            osb = spo.tile([128, td], f16, tag="osb")
                            nc.scalar.activation(
                                osb[:], hp[:],
                                mybir.ActivationFunctionType.Identity,
                                bias=bias[:])
                            nc.sync.dma_start(
                                out_t[:, t * td:(t + 1) * td], osb[:])
                        else:
                            sg = spo.tile([128, td], f16, tag="sg")
                            nc.scalar.activation(
                                sg[:], hp[:],
                                mybir.ActivationFunctionType.Sigmoid,
                                bias=bias[:])
                            hT_t = hT_res[:, t * td:(t + 1) * td]
                            nc.vector.scalar_tensor_tensor(
                                hT_t, hp[:], bias[:], sg[:],
                                mybir.AluOpType.add, mybir.AluOpType.mult)
                            # transpose to row-major and store for tables
                            hr = hpo.tile([128, nblk, 128], f16, tag="hr")
                            for a in range(nblk):
                                tp = psb.tile([128, 128], f16, tag="pst")
                                nc.tensor.transpose(
                                    tp[:], hT_t[:, a * 128:(a + 1) * 128],
                                    ident_sb[:])
                                nc.scalar.copy(hr[:, a, :], tp[:])
                            si_ = nc.sync.dma_start(
                                h_shard[t * td:(t + 1) * td, :].rearrange(
                                    "(a p) f -> p a f", p=128),
                                hr[:])
                            store_by_tile[t] = si_
                return store_by_tile, gather_insts

            prev_l2_gathers = None
            for _ in range(iters):
                l1_stores, _ = layer(x_tabs, w_sb["w1lt"], w_sb["w1rt"],
                                     b_sb["b1"], False, None)
                if SKIP_AG:
                    ag_insts = None
                else:
                    ag_insts = []
                    for q in range(4):
                        ag = nc.gpsimd.collective_compute(
                            "AllGather", mybir.AluOpType.bypass,
                            replica_groups=[list(range(n_cores))],
                            ins=[h_shard[q * quart:(q + 1) * quart, :]],
                            outs=[h_tabs[q][:]])
                        for t in qtiles[q]:
                            if t in l1_stores:
                                add_dep_helper(ag.ins, l1_stores[t].ins,
                                               sync=True,
                                               reason="AG after quarter stores")
                        if prev_l2_gathers is not None:
                            for gi in prev_l2_gathers:
                                add_dep_helper(ag.ins, gi.ins, sync=True,
                                               reason="AG after prev-iter reads")
                        ag_insts.append(ag)
                _, l2_gathers = layer(h_tabs, w_sb["w2lt"], w_sb["w2rt"],
                                      b_sb["b2"], True, ag_insts)
                prev_l2_gathers = l2_gathers

    nc.compile()
    return nc


# ------------------------------------------------------------------ entry
def _prepare(edge_index):
    return _plan(edge_index, N_NODES, NC, TD, STILE)


def _in_maps(plan, x, w1l, w1r, b1, w2l, w2r, b2):
    x16 = np.asarray(x, dtype=np.float16)
    tabs = _make_tables(x16, plan)
    nloc, nloc_pad = plan["nloc"], plan["nloc_pad"]
    n_cores = plan["n_cores"]
    td = plan["td"]; nt = plan["nt"]
    iota = np.broadcast_to(np.arange(td, dtype=np.float16), (128, td)).copy()
    ident = np.eye(128, dtype=np.float16)
    xr = x16.reshape(n_cores, nloc, D)
    maps = []
    for k in range(n_cores):
        xT = np.zeros((128, nloc_pad), np.float16)
        xT[:, :nloc] = xr[k].T
        m = {
            "xT": xT,
            "idxs": _wrap16(plan["idx_st"][k]),
            "drel": _colmajor(plan["drel_st"][k].astype(np.float16)),
            "invb": np.ascontiguousarray(plan["invb"][k], dtype=np.float16),
            "iota": iota, "ident": ident,
            "w1lt": np.ascontiguousarray(np.asarray(w1l, np.float16).T),
            "w1rt": np.ascontiguousarray(np.asarray(w1r, np.float16).T),
            "w2lt": np.ascontiguousarray(np.asarray(w2l, np.float16).T),
            "w2rt": np.ascontiguousarray(np.asarray(w2r, np.float16).T),
            "b1": np.asarray(b1, np.float32).reshape(128, 1),
            "b2": np.asarray(b2, np.float32).reshape(128, 1),
        }
        for q in range(4):
            m[f"x_tab{q}"] = tabs[q]
        maps.append(m)
    return maps


def _run(inputs, iters=1):
    from concourse.bass_utils import run_bass_kernel_spmd

    edge_index = np.asarray(inputs["edge_index"])
    key = ("k2", iters, edge_index.shape[1])
    if key not in _cache:
        plan = _prepare(edge_index)
        nc = _build(plan, iters=iters)
        _cache[key] = (plan, nc)
    plan, nc = _cache[key]
    maps = _in_maps(plan, inputs["x"], inputs["W1_l"], inputs["W1_r"],
                    inputs["b1"], inputs["W2_l"], inputs["W2_r"], inputs["b2"])
    res = run_bass_kernel_spmd(nc, maps, core_ids=list(range(plan["n_cores"])))
    nloc = plan["nloc"]
    outs = [np.asarray(res.results[k]["outT"]).T[:nloc].astype(np.float32)
            for k in range(plan["n_cores"])]
    return np.concatenate(outs, axis=0)


def kernel(**inputs) -> np.ndarray:
    return _run(inputs, iters=1)
